# revision 18
# baseline (speedup 1.0000x reference)
"""Distributed Trainium2 kernel for Informer-style sparse attention (v2).

Math (reference):
    query = emb @ Wq.T + bq ; key = emb @ Wk.T + bk          # [n, d]
    S = query @ key[indices].T                               # [n, 12]
    M = S.max(1); top = top_k(M, 12)
    QK = query[top] @ key.T                                  # [12, n]
    out = QK.max(0) @ emb                                    # [1, d]

Host precompute (f32 numpy, mirrors the baseline's host-side transposes):
    A  = (emb[idx] @ Wk.T + bk) @ Wq ; c = (emb[idx] @ Wk.T + bk) @ bq
        -> S  = emb @ A.T + c        (pass 1, device)
    W2 = Wq.T @ Wk ; b2 = bq @ Wk ; w2b = Wq.T @ bk ; s2 = bq.bk
        -> B  = emb[top] @ W2 + b2 ; c2 = emb[top] @ w2b + s2
        -> QK = B @ emb.T + c2       (pass 2, device)

Device strategy (per core, 8192-row shard):
  - embT shard (16MB bf16) is streamed ONCE into resident SBUF during
    pass 1 and reused for pass 2 (QK) and the final matvec -> total HBM
    traffic ~18MB vs 58MB in v1.
  - final matvec out = pooled @ emb runs on DVE via fused
    tensor_tensor_reduce against the resident embT (no natural-layout
    re-stream).
  - top-k uses value+index PACKING: f32 M truncated to its top 16 bits,
    global token id in the low 16 bits.  Selection = plain MAX8 rounds on
    the packed f32s; the winning ids pop out with a single AND.  No index
    bookkeeping, no DRAM gid bounce, no [1,2048] single-partition ops.
  - 2 collectives: AllGather of 16 packed candidates (64B), final
    AllReduce of [1,1024] f32.
"""

import numpy as np
import ml_dtypes

N = 65536
D = 1024
PICK = 12
NCORES = 8
LOC = N // NCORES          # 8192 rows per core
GRP = 1024                 # tokens per group
NG = LOC // GRP            # 8 groups
NEG = -1.0e30

_cache = {}


def _build():
    import concourse.bass as bass
    import concourse.tile as tile
    import concourse.mybir as mybir
    from concourse import bacc
    from concourse.masks import make_identity

    f32 = mybir.dt.float32
    bf16 = mybir.dt.bfloat16
    i32 = mybir.dt.int32
    u16 = mybir.dt.uint16

    nc = bacc.Bacc("TRN2", target_bir_lowering=False, debug=False,
                   num_devices=NCORES)

    # ---- kernel I/O -------------------------------------------------------
    embT_d = nc.declare_dram_parameter("embT", [D, LOC], bf16, isOutput=False)
    embN_d = nc.declare_dram_parameter("emb_nat", [LOC, D], bf16,
                                       isOutput=False)
    emb_full = nc.declare_dram_parameter("emb_full", [N, D], bf16,
                                         isOutput=False)
    ATc_d = nc.declare_dram_parameter("ATc", [128, 8 * PICK], bf16,
                                      isOutput=False)
    c_d = nc.declare_dram_parameter("c_col", [PICK, 1], f32, isOutput=False)
    W2_d = nc.declare_dram_parameter("W2", [D, D], bf16, isOutput=False)
    b2_d = nc.declare_dram_parameter("b2_row", [1, D], bf16, isOutput=False)
    w2b_d = nc.declare_dram_parameter("w2b_col", [128, 8], bf16,
                                      isOutput=False)
    s2_d = nc.declare_dram_parameter("s2_col", [PICK, 1], f32, isOutput=False)
    gidp_d = nc.declare_dram_parameter("gid_pat", [128, 8 * NG], i32,
                                       isOutput=False)
    out_ext = nc.declare_dram_parameter("out", [1, D], f32, isOutput=True)
    dbg_ext = nc.declare_dram_parameter("dbg", [16, 1], f32, isOutput=True)

    groups = [list(range(NCORES))]

    # collective bounce buffers (internal DRAM)
    warm_in = nc.dram_tensor("warm_in", [16, 1], f32)
    warm_out = nc.dram_tensor("warm_out", [16 * NCORES, 1], f32,
                              addr_space="Shared")
    ag_in = nc.dram_tensor("ag_in", [16, 1], f32)
    ag_out = nc.dram_tensor("ag_out", [16 * NCORES, 1], f32,
                            addr_space="Shared")
    out_cin = nc.dram_tensor("out_cin", [1, D], f32)
    out_cout = nc.dram_tensor("out_cout", [1, D], f32, addr_space="Shared")

    AX = mybir.AxisListType
    ALU = mybir.AluOpType

    with tile.TileContext(nc) as tc:
        with (
            tc.tile_pool(name="persist", bufs=1) as pp,
            tc.tile_pool(name="psA", bufs=3, space="PSUM") as psA,
            tc.tile_pool(name="psT", bufs=2, space="PSUM") as psT,
            tc.tile_pool(name="psacc", bufs=1, space="PSUM") as psacc,
        ):
            # warm up the collective channel during pass 1: the first cc op
            # pays a ~25us bootstrap; burn it on a dummy gather of garbage.
            nc.gpsimd.collective_compute(
                "AllGather", ALU.bypass, replica_groups=groups,
                ins=[warm_in[:, :].opt()], outs=[warm_out[:, :].opt()])

            # ---------- small critical-path loads (gpsimd queue) -----------
            ATc = pp.tile([128, 8 * PICK], bf16)
            nc.gpsimd.dma_start(ATc, ATc_d[:, :])
            c_col = pp.tile([PICK, 1], f32)
            nc.gpsimd.dma_start(c_col, c_d[:, :])
            gid_pat = pp.tile([128, 8 * NG], i32)
            nc.gpsimd.dma_start(gid_pat, gidp_d[:, :])
            w2b_col = pp.tile([128, 8], bf16)
            nc.gpsimd.dma_start(w2b_col, w2b_d[:, :])
            s2_col = pp.tile([PICK, 1], f32)
            nc.gpsimd.dma_start(s2_col, s2_d[:, :])

            # ---------- bulk streams (two HWDGE queues) --------------------
            # embT resident: 8 chunks [128, LOC]; per (g, t) slice DMAs so
            # pass-1 group g can start as soon as its 8 slices land.
            embT = [pp.tile([128, LOC], bf16, name=f"embT{t}", tag=f"embT{t}")
                    for t in range(8)]
            for gq in range(2):
                lo, hi = 4096 * gq, 4096 * (gq + 1)
                for t in range(8):
                    eng = nc.sync if t < 4 else nc.scalar
                    eng.dma_start(embT[t][:, lo:hi],
                                  embT_d[128 * t:128 * (t + 1), lo:hi])
            # W2 + b2 tiles; their DMAs are gated on the AllGather doorbell
            # (below) so the 2MB does not contend with the top-k hop DMAs.
            W2 = [pp.tile([128, D], bf16, name=f"W2_{t}", tag=f"W2_{t}")
                  for t in range(8)]
            b2_row = pp.tile([1, D], bf16)

            # ---------- constants ------------------------------------------
            ident_bf = pp.tile([128, 128], bf16)
            make_identity(nc, ident_bf)
            ones12_bf = pp.tile([1, PICK], bf16)
            nc.vector.memset(ones12_bf, 1.0)
            one1_f = pp.tile([1, 1], f32)
            nc.vector.memset(one1_f, 1.0)

            # pass-2 stream pool opened BEFORE pass 1: its region must not
            # overlap released pools, so the natural-emb prefetch DMAs carry
            # no pool-alloc dependency and flow during the collective gap.
            sp2_cm = tc.tile_pool(name="work2", bufs=3)
            sp2 = sp2_cm.__enter__()
            # natural-emb stream tiles, manual 2-deep rotation per j-chunk
            enb_tiles = [[sp2.tile([128, D], bf16, name="enb",
                                   tag=f"enb{j}_{p}", bufs=1)
                          for p in range(2)] for j in range(8)]

            # ---------- pass 1: M[tok] = max_k (emb @ A.T + c) -------------
            M_sb = pp.tile([128, 8 * NG], bf16)
            sp1_cm = tc.tile_pool(name="work1", bufs=3)
            sp1 = sp1_cm.__enter__()
            for g in range(NG):
                s_sb = sp1.tile([PICK, GRP], bf16, name="s_sb", tag="s_sb",
                                bufs=3)
                for h in range(2):
                    lo = GRP * g + 512 * h
                    s_ps = psA.tile([PICK, 512], f32, name="s_ps", tag="mm",
                                    bufs=3)
                    for t in range(8):
                        nc.tensor.matmul(s_ps, lhsT=ATc[:, PICK * t:PICK * (t + 1)],
                                         rhs=embT[t][:, lo:lo + 512],
                                         start=(t == 0), stop=(t == 7))
                    nc.vector.tensor_scalar(out=s_sb[:, 512 * h:512 * (h + 1)],
                                            in0=s_ps, scalar1=c_col[:, :1],
                                            scalar2=None, op0=ALU.add)
                mt_ps = psT.tile([128, 8, PICK], bf16, name="mt_ps", tag="mt",
                                 bufs=2)
                for j in range(8):
                    nc.tensor.transpose(mt_ps[:, j, :],
                                        s_sb[:, 128 * j:128 * (j + 1)],
                                        ident_bf[:PICK, :PICK])
                nc.vector.tensor_reduce(out=M_sb[:, 8 * g:8 * (g + 1)],
                                        in_=mt_ps[:, :, :], axis=AX.X,
                                        op=ALU.max)
            sp1_cm.__exit__(None, None, None)

            # ---------- pack M (hi 16 bits) | gid (lo 16 bits) -------------
            packed = pp.tile([128, 8 * NG], i32)
            nc.vector.tensor_copy(packed, M_sb[:, :].bitcast(u16))
            nc.vector.tensor_scalar(out=packed, in0=packed,
                                    scalar1=16, scalar2=None,
                                    op0=ALU.logical_shift_left)
            nc.vector.tensor_tensor(out=packed, in0=packed, in1=gid_pat,
                                    op=ALU.bitwise_or)
            pf = packed[:, :].bitcast(f32)

            # ---------- local top-16 (packed: ids ride along) --------------
            t16 = pp.tile([128, 16], f32)
            m_rem = pp.tile([128, 8 * NG], f32)
            nc.vector.max(t16[:, 0:8], pf)
            nc.vector.match_replace(out=m_rem, in_to_replace=t16[:, 0:8],
                                    in_values=pf, imm_value=NEG)
            nc.vector.max(t16[:, 8:16], m_rem)
            # 2048 -> 256 candidates: reshuffle across partitions (any
            # element permutation is fine -- ids travel inside the values)
            fl1 = pp.tile([16, 128], f32)
            nc.gpsimd.dma_start(fl1, t16[:, :])
            t16b = pp.tile([16, 16], f32)
            fl1_rem = pp.tile([16, 128], f32)
            nc.vector.max(t16b[:, 0:8], fl1)
            nc.vector.match_replace(out=fl1_rem, in_to_replace=t16b[:, 0:8],
                                    in_values=fl1, imm_value=NEG)
            nc.vector.max(t16b[:, 8:16], fl1_rem)
            # 256 -> 16
            fl2 = pp.tile([1, 256], f32)
            nc.gpsimd.dma_start(fl2, t16b[:, :])
            vg = pp.tile([1, 16], f32)
            fl2_rem = pp.tile([1, 256], f32)
            nc.vector.max(vg[:, 0:8], fl2)
            nc.vector.match_replace(out=fl2_rem, in_to_replace=vg[:, 0:8],
                                    in_values=fl2, imm_value=NEG)
            nc.vector.max(vg[:, 8:16], fl2_rem)

            # Gate the bulk W2 / natural-emb streams on vg (the last local
            # top-k tile): their 2KB packets otherwise delay the small top-k
            # hop DMAs above by ~5us each.  A 1-element copy sourced from vg
            # into each destination tile makes the real DMA (a later writer
            # of an overlapping range) wait until the chain is done; the
            # streams then run concurrently with the AllGather.
            for t in range(8):
                nc.vector.tensor_copy(W2[t][0:1, 0:1], vg[:, 0:1])
            nc.vector.tensor_copy(b2_row[:, 0:1], vg[:, 0:1])
            for j in range(8):
                for p in range(2):
                    nc.vector.tensor_copy(enb_tiles[j][p][0:1, 0:1],
                                          vg[:, 0:1])
            for t in range(8):
                eng = nc.sync if t < 4 else nc.scalar
                eng.dma_start(W2[t], W2_d[128 * t:128 * (t + 1), :])
            nc.scalar.dma_start(b2_row, b2_d[:, :])
            nc.gpsimd.dma_start(ag_in[:, :], vg)
            nc.gpsimd.collective_compute(
                "AllGather", ALU.bypass, replica_groups=groups,
                ins=[ag_in[:, :].opt()], outs=[ag_out[:, :].opt()])

            # ---------- global top-12 --------------------------------------
            vf = pp.tile([1, 16 * NCORES], f32)
            nc.gpsimd.dma_start(vf, ag_out[:, :])
            vt = pp.tile([1, 16], f32)
            vf_rem = pp.tile([1, 16 * NCORES], f32)
            nc.vector.max(vt[:, 0:8], vf)
            nc.vector.match_replace(out=vf_rem, in_to_replace=vt[:, 0:8],
                                    in_values=vf, imm_value=NEG)
            nc.vector.max(vt[:, 8:16], vf_rem)
            gidu = pp.tile([1, 16], i32)
            vt_ps = psT.tile([16, 1], f32, name="vt_ps", tag="c2", bufs=1)
            nc.tensor.transpose(vt_ps, vt, one1_f[:1, :1])
            vt_col = pp.tile([16, 1], f32)
            nc.vector.tensor_copy(vt_col, vt_ps)
            gid_col = pp.tile([16, 1], i32)
            nc.vector.tensor_scalar(out=gid_col,
                                    in0=vt_col[:, :].bitcast(i32),
                                    scalar1=0xFFFF, scalar2=None,
                                    op0=ALU.bitwise_and)
            embR = pp.tile([16, D], bf16)
            nc.gpsimd.indirect_dma_start(
                out=embR[:, :], out_offset=None, in_=emb_full[:, :],
                in_offset=bass.IndirectOffsetOnAxis(ap=gid_col[:, :1], axis=0))
            dbgf = pp.tile([16, 1], f32)
            nc.vector.tensor_copy(dbgf, gid_col)
            nc.gpsimd.dma_start(dbg_ext[:, :], dbgf)

            # ---------- B-chain: B = embR @ W2 + b2 ; c2 = embR@w2b + s2 ---
            embRT = []
            for t in range(8):
                ps3 = psT.tile([128, 8, PICK], bf16, name="rT_ps", tag="mt",
                               bufs=2)
                ps = ps3[:, 0, :]
                nc.tensor.transpose(ps, embR[:PICK, 128 * t:128 * (t + 1)],
                                    ident_bf[:PICK, :PICK])
                sb = pp.tile([128, PICK], bf16, name=f"embRT{t}",
                             tag=f"embRT{t}")
                nc.vector.tensor_copy(sb, ps)
                embRT.append(sb)
            B_sb = pp.tile([PICK, D], bf16)
            for h in range(2):
                ps = psA.tile([PICK, 512], f32, name="s_ps", tag="mm", bufs=3)
                for t in range(8):
                    nc.tensor.matmul(ps, lhsT=embRT[t],
                                     rhs=W2[t][:, 512 * h:512 * (h + 1)],
                                     start=(t == 0), stop=False)
                nc.tensor.matmul(ps, lhsT=ones12_bf,
                                 rhs=b2_row[:, 512 * h:512 * (h + 1)],
                                 start=False, stop=True)
                nc.vector.tensor_copy(B_sb[:, 512 * h:512 * (h + 1)], ps)
            c2_ps = psT.tile([PICK, 1], f32, name="c2_ps", tag="c2", bufs=1)
            for t in range(8):
                nc.tensor.matmul(c2_ps, lhsT=embRT[t], rhs=w2b_col[:, t:t + 1],
                                 start=(t == 0), stop=(t == 7))
            c2_col = pp.tile([PICK, 1], f32)
            nc.vector.tensor_scalar(out=c2_col, in0=c2_ps,
                                    scalar1=s2_col[:, :1], scalar2=None,
                                    op0=ALU.add)
            BT = []
            for t in range(8):
                ps3 = psT.tile([128, 8, PICK], bf16, name="rT_ps", tag="mt",
                               bufs=2)
                ps = ps3[:, 0, :]
                nc.tensor.transpose(ps, B_sb[:, 128 * t:128 * (t + 1)],
                                    ident_bf[:PICK, :PICK])
                sb = pp.tile([128, PICK], bf16, name=f"BT{t}", tag=f"BT{t}")
                nc.vector.tensor_copy(sb, ps)
                BT.append(sb)

            # ---------- pass 2: QK from resident embT; out accumulated on
            # PE against the natural-layout emb stream (starts prefetching
            # during the collective gap).
            out_ps0 = psacc.tile([1, 512], f32, name="acc0", tag="acc0")
            out_ps1 = psacc.tile([1, 512], f32, name="acc1", tag="acc1")

            ws_n = [0]

            def do_ws(pooled_nat, enbs):
                for j in range(8):
                    first = ws_n[0] == 0
                    last = ws_n[0] == NG * 8 - 1
                    nc.tensor.matmul(out_ps0, lhsT=pooled_nat[:, j:j + 1],
                                     rhs=enbs[j][:, 0:512],
                                     start=first, stop=last)
                    nc.tensor.matmul(out_ps1, lhsT=pooled_nat[:, j:j + 1],
                                     rhs=enbs[j][:, 512:D],
                                     start=first, stop=last)
                    ws_n[0] += 1

            pend = None
            for g in range(NG):
                enbs = []
                for j in range(8):
                    enb = enb_tiles[j][g % 2]
                    eng = nc.sync if j < 4 else nc.scalar
                    eng.dma_start(
                        enb,
                        embN_d[GRP * g + 128 * j:GRP * g + 128 * (j + 1), :])
                    enbs.append(enb)
                s2_sb = sp2.tile([PICK, GRP], bf16, name="s2_sb", tag="s2_sb",
                                 bufs=3)
                for h in range(2):
                    lo = GRP * g + 512 * h
                    s2_ps = psA.tile([PICK, 512], f32, name="s_ps", tag="mm",
                                     bufs=3)
                    for t in range(8):
                        nc.tensor.matmul(s2_ps, lhsT=BT[t],
                                         rhs=embT[t][:, lo:lo + 512],
                                         start=(t == 0), stop=(t == 7))
                    nc.vector.tensor_scalar(out=s2_sb[:, 512 * h:512 * (h + 1)],
                                            in0=s2_ps, scalar1=c2_col[:, :1],
                                            scalar2=None, op0=ALU.add)
                p_ps = psT.tile([128, 8, PICK], bf16, name="p_ps", tag="mt",
                                bufs=2)
                for j in range(8):
                    nc.tensor.transpose(p_ps[:, j, :],
                                        s2_sb[:, 128 * j:128 * (j + 1)],
                                        ident_bf[:PICK, :PICK])
                pooled_nat = sp2.tile([128, 8], bf16, name="pn", tag="pn",
                                      bufs=2)
                nc.vector.tensor_reduce(out=pooled_nat, in_=p_ps[:, :, :],
                                        axis=AX.X, op=ALU.max)
                if pend is not None:
                    do_ws(*pend)
                pend = (pooled_nat, enbs)
            do_ws(*pend)

            out_sb = pp.tile([1, D], f32)
            nc.vector.tensor_copy(out_sb[:, 0:512], out_ps0)
            nc.vector.tensor_copy(out_sb[:, 512:D], out_ps1)
            sp2_cm.__exit__(None, None, None)

            nc.gpsimd.dma_start(out_cin[:, :], out_sb)
            nc.gpsimd.collective_compute(
                "AllReduce", ALU.add, replica_groups=groups,
                ins=[out_cin[:, :].opt()], outs=[out_cout[:, :].opt()])
            nc.gpsimd.dma_start(out_ext[:, :], out_cout[:, :])

    nc.compile()
    return nc


def _in_maps(inputs):
    bf = ml_dtypes.bfloat16
    emb = np.ascontiguousarray(inputs["embed_matrix"], dtype=np.float32)
    Wq = np.ascontiguousarray(inputs["Wq"], dtype=np.float32)
    Wk = np.ascontiguousarray(inputs["Wk"], dtype=np.float32)
    bq = np.ascontiguousarray(inputs["bq"], dtype=np.float32)
    bk = np.ascontiguousarray(inputs["bk"], dtype=np.float32)
    idx = np.ascontiguousarray(inputs["indices"], dtype=np.int64)

    # host-side projections (f32)
    nk = emb[idx] @ Wk.T + bk                       # [12, D]
    A = (nk @ Wq).astype(np.float32)                # S = emb @ A.T + c
    c = (nk @ bq).astype(np.float32)
    W2 = (Wq.T @ Wk).astype(np.float32)             # B = embR @ W2 + b2
    b2 = (bq @ Wk).astype(np.float32)
    w2b = (Wq.T @ bk).astype(np.float32)            # c2 = embR @ w2b + s2
    s2 = np.float32(bq @ bk)

    ATc = np.ascontiguousarray(
        A.T.reshape(8, 128, PICK).transpose(1, 0, 2).reshape(128, 8 * PICK)
    ).astype(bf)
    w2b_col = np.ascontiguousarray(w2b.reshape(8, 128).T).astype(bf)

    emb_full_bf = emb.astype(bf)
    shared = {
        "emb_full": emb_full_bf,
        "ATc": ATc,
        "c_col": c.reshape(PICK, 1),
        "W2": W2.astype(bf),
        "b2_row": b2.reshape(1, D).astype(bf),
        "w2b_col": w2b_col,
        "s2_col": np.full((PICK, 1), s2, dtype=np.float32),
    }
    p = np.arange(128, dtype=np.int32).reshape(128, 1)
    col = np.arange(8 * NG, dtype=np.int32).reshape(1, 8 * NG)
    maps = []
    for cix in range(NCORES):
        m = dict(shared)
        m["embT"] = np.ascontiguousarray(
            emb_full_bf[cix * LOC:(cix + 1) * LOC].T)
        m["emb_nat"] = emb_full_bf[cix * LOC:(cix + 1) * LOC]
        m["gid_pat"] = (cix * LOC + 128 * col + p).astype(np.int32)
        maps.append(m)
    return maps


def kernel(**inputs) -> np.ndarray:
    from concourse.bass_utils import run_bass_kernel_spmd

    if "nc" not in _cache:
        _cache["nc"] = _build()
    nc = _cache["nc"]
    maps = _in_maps(inputs)
    res = run_bass_kernel_spmd(nc, maps, core_ids=list(range(NCORES)))
    _cache["res"] = res
    return np.asarray(res.results[0]["out"], dtype=np.float32)


# revision 19
# speedup vs baseline: 4.9001x; 4.9001x over previous
"""Distributed Trainium2 kernel for Informer-style sparse attention (v2).

Math (reference):
    query = emb @ Wq.T + bq ; key = emb @ Wk.T + bk          # [n, d]
    S = query @ key[indices].T                               # [n, 12]
    M = S.max(1); top = top_k(M, 12)
    QK = query[top] @ key.T                                  # [12, n]
    out = QK.max(0) @ emb                                    # [1, d]

Host precompute (f32 numpy, mirrors the baseline's host-side transposes):
    A  = (emb[idx] @ Wk.T + bk) @ Wq ; c = (emb[idx] @ Wk.T + bk) @ bq
        -> S  = emb @ A.T + c        (pass 1, device)
    W2 = Wq.T @ Wk ; b2 = bq @ Wk ; w2b = Wq.T @ bk ; s2 = bq.bk
        -> B  = emb[top] @ W2 + b2 ; c2 = emb[top] @ w2b + s2
        -> QK = B @ emb.T + c2       (pass 2, device)

Device strategy (per core, 8192-row shard):
  - embT shard (16MB bf16) is streamed ONCE into resident SBUF during
    pass 1 and reused for pass 2 (QK) and the final matvec -> total HBM
    traffic ~18MB vs 58MB in v1.
  - final matvec out = pooled @ emb runs on DVE via fused
    tensor_tensor_reduce against the resident embT (no natural-layout
    re-stream).
  - top-k uses value+index PACKING: f32 M truncated to its top 16 bits,
    global token id in the low 16 bits.  Selection = plain MAX8 rounds on
    the packed f32s; the winning ids pop out with a single AND.  No index
    bookkeeping, no DRAM gid bounce, no [1,2048] single-partition ops.
  - 2 collectives: AllGather of 16 packed candidates (64B), final
    AllReduce of [1,1024] f32.
"""

import numpy as np
import ml_dtypes

N = 65536
D = 1024
PICK = 12
NCORES = 8
LOC = N // NCORES          # 8192 rows per core
GRP = 1024                 # tokens per group
NG = LOC // GRP            # 8 groups
NEG = -1.0e30

_cache = {}


def _build():
    import concourse.bass as bass
    import concourse.tile as tile
    import concourse.mybir as mybir
    from concourse import bacc
    from concourse.masks import make_identity

    f32 = mybir.dt.float32
    bf16 = mybir.dt.bfloat16
    i32 = mybir.dt.int32
    u16 = mybir.dt.uint16

    nc = bacc.Bacc("TRN2", target_bir_lowering=False, debug=False,
                   num_devices=NCORES)

    # ---- kernel I/O -------------------------------------------------------
    embT_d = nc.declare_dram_parameter("embT", [D, LOC], bf16, isOutput=False)
    embN_d = nc.declare_dram_parameter("emb_nat", [LOC, D], bf16,
                                       isOutput=False)
    emb_full = nc.declare_dram_parameter("emb_full", [N, D], bf16,
                                         isOutput=False)
    ATc_d = nc.declare_dram_parameter("ATc", [128, 8 * PICK], bf16,
                                      isOutput=False)
    c_d = nc.declare_dram_parameter("c_col", [PICK, 1], f32, isOutput=False)
    W2_d = nc.declare_dram_parameter("W2", [D, D], bf16, isOutput=False)
    b2_d = nc.declare_dram_parameter("b2_row", [1, D], bf16, isOutput=False)
    w2b_d = nc.declare_dram_parameter("w2b_col", [128, 8], bf16,
                                      isOutput=False)
    s2_d = nc.declare_dram_parameter("s2_col", [PICK, 1], f32, isOutput=False)
    gidp_d = nc.declare_dram_parameter("gid_pat", [128, 8 * NG], i32,
                                       isOutput=False)
    out_ext = nc.declare_dram_parameter("out", [1, D], f32, isOutput=True)
    dbg_ext = nc.declare_dram_parameter("dbg", [16, 1], f32, isOutput=True)

    groups = [list(range(NCORES))]

    # collective bounce buffers (internal DRAM)
    warm_in = nc.dram_tensor("warm_in", [16, 1], f32)
    warm_out = nc.dram_tensor("warm_out", [16 * NCORES, 1], f32,
                              addr_space="Shared")
    ag_in = nc.dram_tensor("ag_in", [16, 1], f32)
    ag_out = nc.dram_tensor("ag_out", [16 * NCORES, 1], f32,
                            addr_space="Shared")
    out_cin = nc.dram_tensor("out_cin", [1, D], f32)
    out_cout = nc.dram_tensor("out_cout", [1, D], f32, addr_space="Shared")

    AX = mybir.AxisListType
    ALU = mybir.AluOpType

    with tile.TileContext(nc) as tc:
        with (
            tc.tile_pool(name="persist", bufs=1) as pp,
            tc.tile_pool(name="psA", bufs=3, space="PSUM") as psA,
            tc.tile_pool(name="psT", bufs=2, space="PSUM") as psT,
            tc.tile_pool(name="psacc", bufs=1, space="PSUM") as psacc,
        ):
            # warm up the collective channel during pass 1: the first cc op
            # pays a ~25us bootstrap; burn it on a dummy gather of garbage.
            nc.gpsimd.collective_compute(
                "AllGather", ALU.bypass, replica_groups=groups,
                ins=[warm_in[:, :].opt()], outs=[warm_out[:, :].opt()])

            # ---------- small critical-path loads (gpsimd queue) -----------
            ATc = pp.tile([128, 8 * PICK], bf16)
            nc.gpsimd.dma_start(ATc, ATc_d[:, :])
            c_col = pp.tile([PICK, 1], f32)
            nc.gpsimd.dma_start(c_col, c_d[:, :])
            gid_pat = pp.tile([128, 8 * NG], i32)
            nc.gpsimd.dma_start(gid_pat, gidp_d[:, :])
            w2b_col = pp.tile([128, 8], bf16)
            nc.gpsimd.dma_start(w2b_col, w2b_d[:, :])
            s2_col = pp.tile([PICK, 1], f32)
            nc.gpsimd.dma_start(s2_col, s2_d[:, :])

            # ---------- bulk streams (two HWDGE queues) --------------------
            # embT resident: 8 chunks [128, LOC]; per (g, t) slice DMAs so
            # pass-1 group g can start as soon as its 8 slices land.
            embT = [pp.tile([128, LOC], bf16, name=f"embT{t}", tag=f"embT{t}")
                    for t in range(8)]
            # front half in 4096-wide slabs (8KB descriptors sustain
            # ~420GB/s vs ~300 at 2KB); back half in 1024-wide slices so the
            # final groups' compute pipelines at fine granularity.
            for t in range(8):
                eng = nc.sync if t < 4 else nc.scalar
                eng.dma_start(embT[t][:, 0:4096],
                              embT_d[128 * t:128 * (t + 1), 0:4096])
            for g in range(4, NG):
                lo, hi = GRP * g, GRP * (g + 1)
                for t in range(8):
                    eng = nc.sync if t < 4 else nc.scalar
                    eng.dma_start(embT[t][:, lo:hi],
                                  embT_d[128 * t:128 * (t + 1), lo:hi])
            # W2 + b2 tiles; their DMAs are gated on the AllGather doorbell
            # (below) so the 2MB does not contend with the top-k hop DMAs.
            W2 = [pp.tile([128, D], bf16, name=f"W2_{t}", tag=f"W2_{t}")
                  for t in range(8)]
            b2_row = pp.tile([1, D], bf16)

            # ---------- constants ------------------------------------------
            ident_bf = pp.tile([128, 128], bf16)
            make_identity(nc, ident_bf)
            ones12_bf = pp.tile([1, PICK], bf16)
            nc.vector.memset(ones12_bf, 1.0)
            one1_f = pp.tile([1, 1], f32)
            nc.vector.memset(one1_f, 1.0)

            # pass-2 stream pool opened BEFORE pass 1: its region must not
            # overlap released pools, so the natural-emb prefetch DMAs carry
            # no pool-alloc dependency and flow during the collective gap.
            sp2_cm = tc.tile_pool(name="work2", bufs=3)
            sp2 = sp2_cm.__enter__()
            # natural-emb stream tiles, manual 2-deep rotation per j-chunk
            enb_tiles = [[sp2.tile([128, D], bf16, name="enb",
                                   tag=f"enb{j}_{p}", bufs=1)
                          for p in range(2)] for j in range(8)]

            # ---------- pass 1: M[tok] = max_k (emb @ A.T + c) -------------
            M_sb = pp.tile([128, 8 * NG], bf16)
            sp1_cm = tc.tile_pool(name="work1", bufs=3)
            sp1 = sp1_cm.__enter__()
            for g in range(NG):
                s_sb = sp1.tile([PICK, GRP], bf16, name="s_sb", tag="s_sb",
                                bufs=3)
                for h in range(2):
                    lo = GRP * g + 512 * h
                    s_ps = psA.tile([PICK, 512], f32, name="s_ps", tag="mm",
                                    bufs=3)
                    for t in range(8):
                        nc.tensor.matmul(s_ps, lhsT=ATc[:, PICK * t:PICK * (t + 1)],
                                         rhs=embT[t][:, lo:lo + 512],
                                         start=(t == 0), stop=(t == 7))
                    nc.vector.tensor_scalar(out=s_sb[:, 512 * h:512 * (h + 1)],
                                            in0=s_ps, scalar1=c_col[:, :1],
                                            scalar2=None, op0=ALU.add)
                mt_ps = psT.tile([128, 8, PICK], bf16, name="mt_ps", tag="mt",
                                 bufs=2)
                for j in range(8):
                    nc.tensor.transpose(mt_ps[:, j, :],
                                        s_sb[:, 128 * j:128 * (j + 1)],
                                        ident_bf[:PICK, :PICK])
                nc.vector.tensor_reduce(out=M_sb[:, 8 * g:8 * (g + 1)],
                                        in_=mt_ps[:, :, :], axis=AX.X,
                                        op=ALU.max)
            sp1_cm.__exit__(None, None, None)

            # ---------- pack M (hi 16 bits) | gid (lo 16 bits) -------------
            packed = pp.tile([128, 8 * NG], i32)
            nc.vector.tensor_copy(packed, M_sb[:, :].bitcast(u16))
            nc.vector.tensor_scalar(out=packed, in0=packed,
                                    scalar1=16, scalar2=None,
                                    op0=ALU.logical_shift_left)
            nc.vector.tensor_tensor(out=packed, in0=packed, in1=gid_pat,
                                    op=ALU.bitwise_or)
            pf = packed[:, :].bitcast(f32)

            # ---------- local top-16 (packed: ids ride along) --------------
            t16 = pp.tile([128, 16], f32)
            m_rem = pp.tile([128, 8 * NG], f32)
            nc.vector.max(t16[:, 0:8], pf)
            nc.vector.match_replace(out=m_rem, in_to_replace=t16[:, 0:8],
                                    in_values=pf, imm_value=NEG)
            nc.vector.max(t16[:, 8:16], m_rem)
            # 2048 -> 256 candidates: reshuffle across partitions (any
            # element permutation is fine -- ids travel inside the values)
            fl1 = pp.tile([16, 128], f32)
            nc.gpsimd.dma_start(fl1, t16[:, :])
            t16b = pp.tile([16, 16], f32)
            fl1_rem = pp.tile([16, 128], f32)
            nc.vector.max(t16b[:, 0:8], fl1)
            nc.vector.match_replace(out=fl1_rem, in_to_replace=t16b[:, 0:8],
                                    in_values=fl1, imm_value=NEG)
            nc.vector.max(t16b[:, 8:16], fl1_rem)
            # 256 -> 16
            fl2 = pp.tile([1, 256], f32)
            nc.gpsimd.dma_start(fl2, t16b[:, :])
            vg = pp.tile([1, 16], f32)
            fl2_rem = pp.tile([1, 256], f32)
            nc.vector.max(vg[:, 0:8], fl2)
            nc.vector.match_replace(out=fl2_rem, in_to_replace=vg[:, 0:8],
                                    in_values=fl2, imm_value=NEG)
            nc.vector.max(vg[:, 8:16], fl2_rem)

            # Gate the bulk W2 / natural-emb streams on vg (the last local
            # top-k tile): their 2KB packets otherwise delay the small top-k
            # hop DMAs above by ~5us each.  A 1-element copy sourced from vg
            # into each destination tile makes the real DMA (a later writer
            # of an overlapping range) wait until the chain is done; the
            # streams then run concurrently with the AllGather.
            for t in range(8):
                nc.vector.tensor_copy(W2[t][0:1, 0:1], vg[:, 0:1])
            nc.vector.tensor_copy(b2_row[:, 0:1], vg[:, 0:1])
            for j in range(8):
                for p in range(2):
                    nc.vector.tensor_copy(enb_tiles[j][p][0:1, 0:1],
                                          vg[:, 0:1])
            for t in range(8):
                eng = nc.sync if t < 4 else nc.scalar
                eng.dma_start(W2[t], W2_d[128 * t:128 * (t + 1), :])
            nc.scalar.dma_start(b2_row, b2_d[:, :])
            nc.gpsimd.dma_start(ag_in[:, :], vg)
            nc.gpsimd.collective_compute(
                "AllGather", ALU.bypass, replica_groups=groups,
                ins=[ag_in[:, :].opt()], outs=[ag_out[:, :].opt()])

            # ---------- global top-12 --------------------------------------
            vf = pp.tile([1, 16 * NCORES], f32)
            nc.gpsimd.dma_start(vf, ag_out[:, :])
            vt = pp.tile([1, 16], f32)
            vf_rem = pp.tile([1, 16 * NCORES], f32)
            nc.vector.max(vt[:, 0:8], vf)
            nc.vector.match_replace(out=vf_rem, in_to_replace=vt[:, 0:8],
                                    in_values=vf, imm_value=NEG)
            nc.vector.max(vt[:, 8:16], vf_rem)
            gidu = pp.tile([1, 16], i32)
            vt_ps = psT.tile([16, 1], f32, name="vt_ps", tag="c2", bufs=1)
            nc.tensor.transpose(vt_ps, vt, one1_f[:1, :1])
            vt_col = pp.tile([16, 1], f32)
            nc.vector.tensor_copy(vt_col, vt_ps)
            gid_col = pp.tile([16, 1], i32)
            nc.vector.tensor_scalar(out=gid_col,
                                    in0=vt_col[:, :].bitcast(i32),
                                    scalar1=0xFFFF, scalar2=None,
                                    op0=ALU.bitwise_and)
            embR = pp.tile([16, D], bf16)
            nc.gpsimd.indirect_dma_start(
                out=embR[:, :], out_offset=None, in_=emb_full[:, :],
                in_offset=bass.IndirectOffsetOnAxis(ap=gid_col[:, :1], axis=0))
            dbgf = pp.tile([16, 1], f32)
            nc.vector.tensor_copy(dbgf, gid_col)
            nc.gpsimd.dma_start(dbg_ext[:, :], dbgf)

            # ---------- B-chain: B = embR @ W2 + b2 ; c2 = embR@w2b + s2 ---
            embRT = []
            for t in range(8):
                ps3 = psT.tile([128, 8, PICK], bf16, name="rT_ps", tag="mt",
                               bufs=2)
                ps = ps3[:, 0, :]
                nc.tensor.transpose(ps, embR[:PICK, 128 * t:128 * (t + 1)],
                                    ident_bf[:PICK, :PICK])
                sb = pp.tile([128, PICK], bf16, name=f"embRT{t}",
                             tag=f"embRT{t}")
                nc.vector.tensor_copy(sb, ps)
                embRT.append(sb)
            B_sb = pp.tile([PICK, D], bf16)
            for h in range(2):
                ps = psA.tile([PICK, 512], f32, name="s_ps", tag="mm", bufs=3)
                for t in range(8):
                    nc.tensor.matmul(ps, lhsT=embRT[t],
                                     rhs=W2[t][:, 512 * h:512 * (h + 1)],
                                     start=(t == 0), stop=False)
                nc.tensor.matmul(ps, lhsT=ones12_bf,
                                 rhs=b2_row[:, 512 * h:512 * (h + 1)],
                                 start=False, stop=True)
                nc.vector.tensor_copy(B_sb[:, 512 * h:512 * (h + 1)], ps)
            c2_ps = psT.tile([PICK, 1], f32, name="c2_ps", tag="c2", bufs=1)
            for t in range(8):
                nc.tensor.matmul(c2_ps, lhsT=embRT[t], rhs=w2b_col[:, t:t + 1],
                                 start=(t == 0), stop=(t == 7))
            c2_col = pp.tile([PICK, 1], f32)
            nc.vector.tensor_scalar(out=c2_col, in0=c2_ps,
                                    scalar1=s2_col[:, :1], scalar2=None,
                                    op0=ALU.add)
            BT = []
            for t in range(8):
                ps3 = psT.tile([128, 8, PICK], bf16, name="rT_ps", tag="mt",
                               bufs=2)
                ps = ps3[:, 0, :]
                nc.tensor.transpose(ps, B_sb[:, 128 * t:128 * (t + 1)],
                                    ident_bf[:PICK, :PICK])
                sb = pp.tile([128, PICK], bf16, name=f"BT{t}", tag=f"BT{t}")
                nc.vector.tensor_copy(sb, ps)
                BT.append(sb)

            # ---------- pass 2: QK from resident embT; out accumulated on
            # PE against the natural-layout emb stream (starts prefetching
            # during the collective gap).
            out_ps0 = psacc.tile([1, 512], f32, name="acc0", tag="acc0")
            out_ps1 = psacc.tile([1, 512], f32, name="acc1", tag="acc1")

            ws_n = [0]

            def do_ws(pooled_nat, enbs):
                for j in range(8):
                    first = ws_n[0] == 0
                    last = ws_n[0] == NG * 8 - 1
                    nc.tensor.matmul(out_ps0, lhsT=pooled_nat[:, j:j + 1],
                                     rhs=enbs[j][:, 0:512],
                                     start=first, stop=last)
                    nc.tensor.matmul(out_ps1, lhsT=pooled_nat[:, j:j + 1],
                                     rhs=enbs[j][:, 512:D],
                                     start=first, stop=last)
                    ws_n[0] += 1

            pend = None
            for g in range(NG):
                enbs = []
                for j in range(8):
                    enb = enb_tiles[j][g % 2]
                    eng = nc.sync if j < 4 else nc.scalar
                    eng.dma_start(
                        enb,
                        embN_d[GRP * g + 128 * j:GRP * g + 128 * (j + 1), :])
                    enbs.append(enb)
                s2_sb = sp2.tile([PICK, GRP], bf16, name="s2_sb", tag="s2_sb",
                                 bufs=3)
                for h in range(2):
                    lo = GRP * g + 512 * h
                    s2_ps = psA.tile([PICK, 512], f32, name="s_ps", tag="mm",
                                     bufs=3)
                    for t in range(8):
                        nc.tensor.matmul(s2_ps, lhsT=BT[t],
                                         rhs=embT[t][:, lo:lo + 512],
                                         start=(t == 0), stop=(t == 7))
                    nc.vector.tensor_scalar(out=s2_sb[:, 512 * h:512 * (h + 1)],
                                            in0=s2_ps, scalar1=c2_col[:, :1],
                                            scalar2=None, op0=ALU.add)
                p_ps = psT.tile([128, 8, PICK], bf16, name="p_ps", tag="mt",
                                bufs=2)
                for j in range(8):
                    nc.tensor.transpose(p_ps[:, j, :],
                                        s2_sb[:, 128 * j:128 * (j + 1)],
                                        ident_bf[:PICK, :PICK])
                pooled_nat = sp2.tile([128, 8], bf16, name="pn", tag="pn",
                                      bufs=2)
                nc.vector.tensor_reduce(out=pooled_nat, in_=p_ps[:, :, :],
                                        axis=AX.X, op=ALU.max)
                if pend is not None:
                    do_ws(*pend)
                pend = (pooled_nat, enbs)
            do_ws(*pend)

            out_sb = pp.tile([1, D], f32)
            nc.vector.tensor_copy(out_sb[:, 0:512], out_ps0)
            nc.vector.tensor_copy(out_sb[:, 512:D], out_ps1)
            sp2_cm.__exit__(None, None, None)

            nc.gpsimd.dma_start(out_cin[:, :], out_sb)
            nc.gpsimd.collective_compute(
                "AllReduce", ALU.add, replica_groups=groups,
                ins=[out_cin[:, :].opt()], outs=[out_cout[:, :].opt()])
            nc.gpsimd.dma_start(out_ext[:, :], out_cout[:, :])

    nc.compile()
    return nc


def _in_maps(inputs):
    bf = ml_dtypes.bfloat16
    emb = np.ascontiguousarray(inputs["embed_matrix"], dtype=np.float32)
    Wq = np.ascontiguousarray(inputs["Wq"], dtype=np.float32)
    Wk = np.ascontiguousarray(inputs["Wk"], dtype=np.float32)
    bq = np.ascontiguousarray(inputs["bq"], dtype=np.float32)
    bk = np.ascontiguousarray(inputs["bk"], dtype=np.float32)
    idx = np.ascontiguousarray(inputs["indices"], dtype=np.int64)

    # host-side projections (f32)
    nk = emb[idx] @ Wk.T + bk                       # [12, D]
    A = (nk @ Wq).astype(np.float32)                # S = emb @ A.T + c
    c = (nk @ bq).astype(np.float32)
    W2 = (Wq.T @ Wk).astype(np.float32)             # B = embR @ W2 + b2
    b2 = (bq @ Wk).astype(np.float32)
    w2b = (Wq.T @ bk).astype(np.float32)            # c2 = embR @ w2b + s2
    s2 = np.float32(bq @ bk)

    ATc = np.ascontiguousarray(
        A.T.reshape(8, 128, PICK).transpose(1, 0, 2).reshape(128, 8 * PICK)
    ).astype(bf)
    w2b_col = np.ascontiguousarray(w2b.reshape(8, 128).T).astype(bf)

    emb_full_bf = emb.astype(bf)
    shared = {
        "emb_full": emb_full_bf,
        "ATc": ATc,
        "c_col": c.reshape(PICK, 1),
        "W2": W2.astype(bf),
        "b2_row": b2.reshape(1, D).astype(bf),
        "w2b_col": w2b_col,
        "s2_col": np.full((PICK, 1), s2, dtype=np.float32),
    }
    p = np.arange(128, dtype=np.int32).reshape(128, 1)
    col = np.arange(8 * NG, dtype=np.int32).reshape(1, 8 * NG)
    maps = []
    for cix in range(NCORES):
        m = dict(shared)
        m["embT"] = np.ascontiguousarray(
            emb_full_bf[cix * LOC:(cix + 1) * LOC].T)
        m["emb_nat"] = emb_full_bf[cix * LOC:(cix + 1) * LOC]
        m["gid_pat"] = (cix * LOC + 128 * col + p).astype(np.int32)
        maps.append(m)
    return maps


def kernel(**inputs) -> np.ndarray:
    from concourse.bass_utils import run_bass_kernel_spmd

    if "nc" not in _cache:
        _cache["nc"] = _build()
    nc = _cache["nc"]
    maps = _in_maps(inputs)
    res = run_bass_kernel_spmd(nc, maps, core_ids=list(range(NCORES)))
    _cache["res"] = res
    return np.asarray(res.results[0]["out"], dtype=np.float32)


# revision 20
# speedup vs baseline: 6.6966x; 1.3666x over previous
"""Distributed Trainium2 kernel for Informer-style sparse attention (v2).

Math (reference):
    query = emb @ Wq.T + bq ; key = emb @ Wk.T + bk          # [n, d]
    S = query @ key[indices].T                               # [n, 12]
    M = S.max(1); top = top_k(M, 12)
    QK = query[top] @ key.T                                  # [12, n]
    out = QK.max(0) @ emb                                    # [1, d]

Host precompute (f32 numpy, mirrors the baseline's host-side transposes):
    A  = (emb[idx] @ Wk.T + bk) @ Wq ; c = (emb[idx] @ Wk.T + bk) @ bq
        -> S  = emb @ A.T + c        (pass 1, device)
    W2 = Wq.T @ Wk ; b2 = bq @ Wk ; w2b = Wq.T @ bk ; s2 = bq.bk
        -> B  = emb[top] @ W2 + b2 ; c2 = emb[top] @ w2b + s2
        -> QK = B @ emb.T + c2       (pass 2, device)

Device strategy (per core, 8192-row shard):
  - embT shard (16MB bf16) is streamed ONCE into resident SBUF during
    pass 1 and reused for pass 2 (QK) and the final matvec -> total HBM
    traffic ~18MB vs 58MB in v1.
  - final matvec out = pooled @ emb runs on DVE via fused
    tensor_tensor_reduce against the resident embT (no natural-layout
    re-stream).
  - top-k uses value+index PACKING: f32 M truncated to its top 16 bits,
    global token id in the low 16 bits.  Selection = plain MAX8 rounds on
    the packed f32s; the winning ids pop out with a single AND.  No index
    bookkeeping, no DRAM gid bounce, no [1,2048] single-partition ops.
  - 2 collectives: AllGather of 16 packed candidates (64B), final
    AllReduce of [1,1024] f32.
"""

import numpy as np
import ml_dtypes

N = 65536
D = 1024
PICK = 12
NCORES = 8
LOC = N // NCORES          # 8192 rows per core
GRP = 1024                 # tokens per group
NG = LOC // GRP            # 8 groups
NEG = -1.0e30

_cache = {}


def _build():
    import concourse.bass as bass
    import concourse.tile as tile
    import concourse.mybir as mybir
    from concourse import bacc
    from concourse.masks import make_identity

    f32 = mybir.dt.float32
    bf16 = mybir.dt.bfloat16
    i32 = mybir.dt.int32
    u16 = mybir.dt.uint16

    nc = bacc.Bacc("TRN2", target_bir_lowering=False, debug=False,
                   num_devices=NCORES)

    # ---- kernel I/O -------------------------------------------------------
    embT_d = nc.declare_dram_parameter("embT", [D, LOC], bf16, isOutput=False)
    embN_d = nc.declare_dram_parameter("emb_nat", [16 * 128, 4 * D], bf16,
                                       isOutput=False)
    emb_full = nc.declare_dram_parameter("emb_full", [N, D], bf16,
                                         isOutput=False)
    ATc_d = nc.declare_dram_parameter("ATc", [128, 8 * PICK], bf16,
                                      isOutput=False)
    c_d = nc.declare_dram_parameter("c_col", [PICK, 1], f32, isOutput=False)
    W2_d = nc.declare_dram_parameter("W2", [D, D], bf16, isOutput=False)
    b2_d = nc.declare_dram_parameter("b2_row", [1, D], bf16, isOutput=False)
    w2b_d = nc.declare_dram_parameter("w2b_col", [128, 8], bf16,
                                      isOutput=False)
    s2_d = nc.declare_dram_parameter("s2_col", [PICK, 1], f32, isOutput=False)
    gidp_d = nc.declare_dram_parameter("gid_pat", [128, 8 * NG], i32,
                                       isOutput=False)
    out_ext = nc.declare_dram_parameter("out", [1, D], f32, isOutput=True)
    dbg_ext = nc.declare_dram_parameter("dbg", [16, 1], f32, isOutput=True)

    groups = [list(range(NCORES))]

    # collective bounce buffers (internal DRAM)
    warm_in = nc.dram_tensor("warm_in", [16, 1], f32)
    warm_out = nc.dram_tensor("warm_out", [16 * NCORES, 1], f32,
                              addr_space="Shared")
    ag_in = nc.dram_tensor("ag_in", [16, 1], f32)
    ag_out = nc.dram_tensor("ag_out", [16 * NCORES, 1], f32,
                            addr_space="Shared")
    out_cin = nc.dram_tensor("out_cin", [1, D], f32)
    out_cout = nc.dram_tensor("out_cout", [1, D], f32, addr_space="Shared")

    AX = mybir.AxisListType
    ALU = mybir.AluOpType

    with tile.TileContext(nc) as tc:
        with (
            tc.tile_pool(name="persist", bufs=1) as pp,
            tc.tile_pool(name="psA", bufs=3, space="PSUM") as psA,
            tc.tile_pool(name="psT", bufs=2, space="PSUM") as psT,
            tc.tile_pool(name="psacc", bufs=1, space="PSUM") as psacc,
        ):
            # warm up the collective channel during pass 1: the first cc op
            # pays a ~25us bootstrap; burn it on a dummy gather of garbage.
            nc.gpsimd.collective_compute(
                "AllGather", ALU.bypass, replica_groups=groups,
                ins=[warm_in[:, :].opt()], outs=[warm_out[:, :].opt()])

            # ---------- small critical-path loads (gpsimd queue) -----------
            ATc = pp.tile([128, 8 * PICK], bf16)
            nc.gpsimd.dma_start(ATc, ATc_d[:, :])
            c_col = pp.tile([PICK, 1], f32)
            nc.gpsimd.dma_start(c_col, c_d[:, :])
            gid_pat = pp.tile([128, 8 * NG], i32)
            nc.gpsimd.dma_start(gid_pat, gidp_d[:, :])
            w2b_col = pp.tile([128, 8], bf16)
            nc.gpsimd.dma_start(w2b_col, w2b_d[:, :])
            s2_col = pp.tile([PICK, 1], f32)
            nc.gpsimd.dma_start(s2_col, s2_d[:, :])

            # ---------- bulk streams (two HWDGE queues) --------------------
            # embT resident: 8 chunks [128, LOC]; per (g, t) slice DMAs so
            # pass-1 group g can start as soon as its 8 slices land.
            embT = [pp.tile([128, LOC], bf16, name=f"embT{t}", tag=f"embT{t}")
                    for t in range(8)]
            # front half in 4096-wide slabs (8KB descriptors sustain
            # ~420GB/s vs ~300 at 2KB); back half in 1024-wide slices so the
            # final groups' compute pipelines at fine granularity.
            for t in range(8):
                eng = nc.sync if t < 4 else nc.scalar
                eng.dma_start(embT[t][:, 0:4096],
                              embT_d[128 * t:128 * (t + 1), 0:4096])
            for g in range(4, NG):
                lo, hi = GRP * g, GRP * (g + 1)
                for t in range(8):
                    eng = nc.sync if t < 4 else nc.scalar
                    eng.dma_start(embT[t][:, lo:hi],
                                  embT_d[128 * t:128 * (t + 1), lo:hi])
            # W2 + b2 tiles; their DMAs are gated on the AllGather doorbell
            # (below) so the 2MB does not contend with the top-k hop DMAs.
            W2 = [pp.tile([128, D], bf16, name=f"W2_{t}", tag=f"W2_{t}")
                  for t in range(8)]
            b2_row = pp.tile([1, D], bf16)

            # ---------- constants ------------------------------------------
            ident_bf = pp.tile([128, 128], bf16)
            make_identity(nc, ident_bf)
            ones12_bf = pp.tile([1, PICK], bf16)
            nc.vector.memset(ones12_bf, 1.0)
            one1_f = pp.tile([1, 1], f32)
            nc.vector.memset(one1_f, 1.0)

            # pass-2 stream pool opened BEFORE pass 1: its region must not
            # overlap released pools, so the natural-emb prefetch DMAs carry
            # no pool-alloc dependency and flow during the collective gap.
            sp2_cm = tc.tile_pool(name="work2", bufs=3)
            sp2 = sp2_cm.__enter__()
            # natural-emb stream tiles: [128, 4096] = 4 row-blocks each so
            # the DRAM lines are 8KB (sustains ~420GB/s vs ~270 at 2KB);
            # manual 2-deep rotation per half-group
            enb_tiles = [[sp2.tile([128, 4 * D], bf16, name="enb",
                                   tag=f"enb4_{h}_{p}", bufs=1)
                          for p in range(2)] for h in range(2)]

            # ---------- pass 1: M[tok] = max_k (emb @ A.T + c) -------------
            M_sb = pp.tile([128, 8 * NG], bf16)
            sp1_cm = tc.tile_pool(name="work1", bufs=3)
            sp1 = sp1_cm.__enter__()
            for g in range(NG):
                s_sb = sp1.tile([PICK, GRP], bf16, name="s_sb", tag="s_sb",
                                bufs=3)
                for h in range(2):
                    lo = GRP * g + 512 * h
                    s_ps = psA.tile([PICK, 512], f32, name="s_ps", tag="mm",
                                    bufs=3)
                    for t in range(8):
                        nc.tensor.matmul(s_ps, lhsT=ATc[:, PICK * t:PICK * (t + 1)],
                                         rhs=embT[t][:, lo:lo + 512],
                                         start=(t == 0), stop=(t == 7))
                    nc.vector.tensor_scalar(out=s_sb[:, 512 * h:512 * (h + 1)],
                                            in0=s_ps, scalar1=c_col[:, :1],
                                            scalar2=None, op0=ALU.add)
                mt_ps = psT.tile([128, 8, PICK], bf16, name="mt_ps", tag="mt",
                                 bufs=2)
                for j in range(8):
                    nc.tensor.transpose(mt_ps[:, j, :],
                                        s_sb[:, 128 * j:128 * (j + 1)],
                                        ident_bf[:PICK, :PICK])
                nc.vector.tensor_reduce(out=M_sb[:, 8 * g:8 * (g + 1)],
                                        in_=mt_ps[:, :, :], axis=AX.X,
                                        op=ALU.max)
            sp1_cm.__exit__(None, None, None)

            # ---------- pack M (hi 16 bits) | gid (lo 16 bits) -------------
            packed = pp.tile([128, 8 * NG], i32)
            nc.vector.tensor_copy(packed, M_sb[:, :].bitcast(u16))
            nc.vector.tensor_scalar(out=packed, in0=packed,
                                    scalar1=16, scalar2=None,
                                    op0=ALU.logical_shift_left)
            nc.vector.tensor_tensor(out=packed, in0=packed, in1=gid_pat,
                                    op=ALU.bitwise_or)
            pf = packed[:, :].bitcast(f32)

            # ---------- local top-16 (packed: ids ride along) --------------
            t16 = pp.tile([128, 16], f32)
            m_rem = pp.tile([128, 8 * NG], f32)
            nc.vector.max(t16[:, 0:8], pf)
            nc.vector.match_replace(out=m_rem, in_to_replace=t16[:, 0:8],
                                    in_values=pf, imm_value=NEG)
            nc.vector.max(t16[:, 8:16], m_rem)
            # 2048 -> 256 candidates: reshuffle across partitions (any
            # element permutation is fine -- ids travel inside the values)
            fl1 = pp.tile([16, 128], f32)
            nc.gpsimd.dma_start(fl1, t16[:, :])
            t16b = pp.tile([16, 16], f32)
            fl1_rem = pp.tile([16, 128], f32)
            nc.vector.max(t16b[:, 0:8], fl1)
            nc.vector.match_replace(out=fl1_rem, in_to_replace=t16b[:, 0:8],
                                    in_values=fl1, imm_value=NEG)
            nc.vector.max(t16b[:, 8:16], fl1_rem)
            # 256 -> 16
            fl2 = pp.tile([1, 256], f32)
            nc.gpsimd.dma_start(fl2, t16b[:, :])
            vg = pp.tile([1, 16], f32)
            fl2_rem = pp.tile([1, 256], f32)
            nc.vector.max(vg[:, 0:8], fl2)
            nc.vector.match_replace(out=fl2_rem, in_to_replace=vg[:, 0:8],
                                    in_values=fl2, imm_value=NEG)
            nc.vector.max(vg[:, 8:16], fl2_rem)

            # Gate the bulk W2 / natural-emb streams on vg (the last local
            # top-k tile): their 2KB packets otherwise delay the small top-k
            # hop DMAs above by ~5us each.  A 1-element copy sourced from vg
            # into each destination tile makes the real DMA (a later writer
            # of an overlapping range) wait until the chain is done; the
            # streams then run concurrently with the AllGather.
            for t in range(8):
                nc.vector.tensor_copy(W2[t][0:1, 0:1], vg[:, 0:1])
            nc.vector.tensor_copy(b2_row[:, 0:1], vg[:, 0:1])
            for h in range(2):
                for p in range(2):
                    nc.vector.tensor_copy(enb_tiles[h][p][0:1, 0:1],
                                          vg[:, 0:1])
            for t in range(8):
                eng = nc.sync if t < 4 else nc.scalar
                eng.dma_start(W2[t], W2_d[128 * t:128 * (t + 1), :])
            nc.scalar.dma_start(b2_row, b2_d[:, :])
            nc.gpsimd.dma_start(ag_in[:, :], vg)
            nc.gpsimd.collective_compute(
                "AllGather", ALU.bypass, replica_groups=groups,
                ins=[ag_in[:, :].opt()], outs=[ag_out[:, :].opt()])

            # ---------- global top-12 --------------------------------------
            vf = pp.tile([1, 16 * NCORES], f32)
            nc.gpsimd.dma_start(vf, ag_out[:, :])
            vt = pp.tile([1, 16], f32)
            vf_rem = pp.tile([1, 16 * NCORES], f32)
            nc.vector.max(vt[:, 0:8], vf)
            nc.vector.match_replace(out=vf_rem, in_to_replace=vt[:, 0:8],
                                    in_values=vf, imm_value=NEG)
            nc.vector.max(vt[:, 8:16], vf_rem)
            gidu = pp.tile([1, 16], i32)
            vt_ps = psT.tile([16, 1], f32, name="vt_ps", tag="c2", bufs=1)
            nc.tensor.transpose(vt_ps, vt, one1_f[:1, :1])
            vt_col = pp.tile([16, 1], f32)
            nc.vector.tensor_copy(vt_col, vt_ps)
            gid_col = pp.tile([16, 1], i32)
            nc.vector.tensor_scalar(out=gid_col,
                                    in0=vt_col[:, :].bitcast(i32),
                                    scalar1=0xFFFF, scalar2=None,
                                    op0=ALU.bitwise_and)
            embR = pp.tile([16, D], bf16)
            nc.gpsimd.indirect_dma_start(
                out=embR[:, :], out_offset=None, in_=emb_full[:, :],
                in_offset=bass.IndirectOffsetOnAxis(ap=gid_col[:, :1], axis=0))
            dbgf = pp.tile([16, 1], f32)
            nc.vector.tensor_copy(dbgf, gid_col)
            nc.gpsimd.dma_start(dbg_ext[:, :], dbgf)

            # ---------- B-chain: B = embR @ W2 + b2 ; c2 = embR@w2b + s2 ---
            embRT = []
            for t in range(8):
                ps3 = psT.tile([128, 8, PICK], bf16, name="rT_ps", tag="mt",
                               bufs=2)
                ps = ps3[:, 0, :]
                nc.tensor.transpose(ps, embR[:PICK, 128 * t:128 * (t + 1)],
                                    ident_bf[:PICK, :PICK])
                sb = pp.tile([128, PICK], bf16, name=f"embRT{t}",
                             tag=f"embRT{t}")
                nc.vector.tensor_copy(sb, ps)
                embRT.append(sb)
            B_sb = pp.tile([PICK, D], bf16)
            for h in range(2):
                ps = psA.tile([PICK, 512], f32, name="s_ps", tag="mm", bufs=3)
                for t in range(8):
                    nc.tensor.matmul(ps, lhsT=embRT[t],
                                     rhs=W2[t][:, 512 * h:512 * (h + 1)],
                                     start=(t == 0), stop=False)
                nc.tensor.matmul(ps, lhsT=ones12_bf,
                                 rhs=b2_row[:, 512 * h:512 * (h + 1)],
                                 start=False, stop=True)
                nc.vector.tensor_copy(B_sb[:, 512 * h:512 * (h + 1)], ps)
            c2_ps = psT.tile([PICK, 1], f32, name="c2_ps", tag="c2", bufs=1)
            for t in range(8):
                nc.tensor.matmul(c2_ps, lhsT=embRT[t], rhs=w2b_col[:, t:t + 1],
                                 start=(t == 0), stop=(t == 7))
            c2_col = pp.tile([PICK, 1], f32)
            nc.vector.tensor_scalar(out=c2_col, in0=c2_ps,
                                    scalar1=s2_col[:, :1], scalar2=None,
                                    op0=ALU.add)
            BT = []
            for t in range(8):
                ps3 = psT.tile([128, 8, PICK], bf16, name="rT_ps", tag="mt",
                               bufs=2)
                ps = ps3[:, 0, :]
                nc.tensor.transpose(ps, B_sb[:, 128 * t:128 * (t + 1)],
                                    ident_bf[:PICK, :PICK])
                sb = pp.tile([128, PICK], bf16, name=f"BT{t}", tag=f"BT{t}")
                nc.vector.tensor_copy(sb, ps)
                BT.append(sb)

            # ---------- pass 2: QK from resident embT; out accumulated on
            # PE against the natural-layout emb stream (starts prefetching
            # during the collective gap).
            out_ps0 = psacc.tile([1, 512], f32, name="acc0", tag="acc0")
            out_ps1 = psacc.tile([1, 512], f32, name="acc1", tag="acc1")

            ws_n = [0]

            def do_ws(pooled_nat, enbs):
                for j in range(8):
                    first = ws_n[0] == 0
                    last = ws_n[0] == NG * 8 - 1
                    base = D * (j % 4)
                    nc.tensor.matmul(out_ps0, lhsT=pooled_nat[:, j:j + 1],
                                     rhs=enbs[j // 4][:, base:base + 512],
                                     start=first, stop=last)
                    nc.tensor.matmul(out_ps1, lhsT=pooled_nat[:, j:j + 1],
                                     rhs=enbs[j // 4][:, base + 512:base + D],
                                     start=first, stop=last)
                    ws_n[0] += 1

            pend = None
            for g in range(NG):
                enbs = []
                for h in range(2):
                    enb = enb_tiles[h][g % 2]
                    eng = nc.sync if h == 0 else nc.scalar
                    row = (2 * g + h) * 128
                    eng.dma_start(enb, embN_d[row:row + 128, :])
                    enbs.append(enb)
                s2_sb = sp2.tile([PICK, GRP], bf16, name="s2_sb", tag="s2_sb",
                                 bufs=3)
                for h in range(2):
                    lo = GRP * g + 512 * h
                    s2_ps = psA.tile([PICK, 512], f32, name="s_ps", tag="mm",
                                     bufs=3)
                    for t in range(8):
                        nc.tensor.matmul(s2_ps, lhsT=BT[t],
                                         rhs=embT[t][:, lo:lo + 512],
                                         start=(t == 0), stop=(t == 7))
                    nc.vector.tensor_scalar(out=s2_sb[:, 512 * h:512 * (h + 1)],
                                            in0=s2_ps, scalar1=c2_col[:, :1],
                                            scalar2=None, op0=ALU.add)
                p_ps = psT.tile([128, 8, PICK], bf16, name="p_ps", tag="mt",
                                bufs=2)
                for j in range(8):
                    nc.tensor.transpose(p_ps[:, j, :],
                                        s2_sb[:, 128 * j:128 * (j + 1)],
                                        ident_bf[:PICK, :PICK])
                pooled_nat = sp2.tile([128, 8], bf16, name="pn", tag="pn",
                                      bufs=2)
                nc.vector.tensor_reduce(out=pooled_nat, in_=p_ps[:, :, :],
                                        axis=AX.X, op=ALU.max)
                if pend is not None:
                    do_ws(*pend)
                pend = (pooled_nat, enbs)
            do_ws(*pend)

            out_sb = pp.tile([1, D], f32)
            nc.vector.tensor_copy(out_sb[:, 0:512], out_ps0)
            nc.vector.tensor_copy(out_sb[:, 512:D], out_ps1)
            sp2_cm.__exit__(None, None, None)

            nc.gpsimd.dma_start(out_cin[:, :], out_sb)
            nc.gpsimd.collective_compute(
                "AllReduce", ALU.add, replica_groups=groups,
                ins=[out_cin[:, :].opt()], outs=[out_cout[:, :].opt()])
            nc.gpsimd.dma_start(out_ext[:, :], out_cout[:, :])

    nc.compile()
    return nc


def _in_maps(inputs):
    bf = ml_dtypes.bfloat16
    emb = np.ascontiguousarray(inputs["embed_matrix"], dtype=np.float32)
    Wq = np.ascontiguousarray(inputs["Wq"], dtype=np.float32)
    Wk = np.ascontiguousarray(inputs["Wk"], dtype=np.float32)
    bq = np.ascontiguousarray(inputs["bq"], dtype=np.float32)
    bk = np.ascontiguousarray(inputs["bk"], dtype=np.float32)
    idx = np.ascontiguousarray(inputs["indices"], dtype=np.int64)

    # host-side projections (f32)
    nk = emb[idx] @ Wk.T + bk                       # [12, D]
    A = (nk @ Wq).astype(np.float32)                # S = emb @ A.T + c
    c = (nk @ bq).astype(np.float32)
    W2 = (Wq.T @ Wk).astype(np.float32)             # B = embR @ W2 + b2
    b2 = (bq @ Wk).astype(np.float32)
    w2b = (Wq.T @ bk).astype(np.float32)            # c2 = embR @ w2b + s2
    s2 = np.float32(bq @ bk)

    ATc = np.ascontiguousarray(
        A.T.reshape(8, 128, PICK).transpose(1, 0, 2).reshape(128, 8 * PICK)
    ).astype(bf)
    w2b_col = np.ascontiguousarray(w2b.reshape(8, 128).T).astype(bf)

    emb_full_bf = emb.astype(bf)
    shared = {
        "emb_full": emb_full_bf,
        "ATc": ATc,
        "c_col": c.reshape(PICK, 1),
        "W2": W2.astype(bf),
        "b2_row": b2.reshape(1, D).astype(bf),
        "w2b_col": w2b_col,
        "s2_col": np.full((PICK, 1), s2, dtype=np.float32),
    }
    p = np.arange(128, dtype=np.int32).reshape(128, 1)
    col = np.arange(8 * NG, dtype=np.int32).reshape(1, 8 * NG)
    maps = []
    for cix in range(NCORES):
        m = dict(shared)
        m["embT"] = np.ascontiguousarray(
            emb_full_bf[cix * LOC:(cix + 1) * LOC].T)
        E = emb_full_bf[cix * LOC:(cix + 1) * LOC]
        m["emb_nat"] = np.ascontiguousarray(
            E.reshape(8, 2, 4, 128, D).transpose(0, 1, 3, 2, 4)
            .reshape(16 * 128, 4 * D))
        m["gid_pat"] = (cix * LOC + 128 * col + p).astype(np.int32)
        maps.append(m)
    return maps


def kernel(**inputs) -> np.ndarray:
    from concourse.bass_utils import run_bass_kernel_spmd

    if "nc" not in _cache:
        _cache["nc"] = _build()
    nc = _cache["nc"]
    maps = _in_maps(inputs)
    res = run_bass_kernel_spmd(nc, maps, core_ids=list(range(NCORES)))
    _cache["res"] = res
    return np.asarray(res.results[0]["out"], dtype=np.float32)
